# revision 21
# baseline (speedup 1.0000x reference)
"""Trainium2 Bass kernel: 16-head attention with LoRA (B=2, N=2048, C=1024).

v2: head-group sharding, no collectives. Core c handles batch c//4 and the
4 heads [4*(c%4), 4*(c%4)+4) over the FULL 2048-row sequence. LoRA is folded
into the qkv/proj weights on the host (W' = W + 2*B@A, exact). Each core
computes q,k,v for its heads, runs attention, and emits the partial output
projection over its 256 local channels; the host sums the 4 partials per
batch and adds the bias.

Pipeline: per (query-chunk, head-pair) block, scores (PE, row-packed K=64
pairs) feed exp (Scalar) feed attnV (PE, interleaved one block behind), so
ScalarE's exp stream and the PE matmul stream overlap continuously.
"""

import os
from contextlib import ExitStack

import numpy as np
import ml_dtypes

import concourse.bass as bass
import concourse.mybir as mybir
import concourse.tile as tile
from concourse.bass_utils import run_bass_kernel_spmd

B, N, C, H, D = 2, 2048, 1024, 16, 64
KT = N // 128    # 16 kv tiles of 128
QC = 4           # query chunks of 512
BF = mybir.dt.bfloat16
F32 = mybir.dt.float32
BLOCKS = [(qc, hp) for qc in range(QC) for hp in range(2)]


def build():
    nc = bass.Bass()
    xT = nc.declare_dram_parameter("xT", [C, N], BF, isOutput=False)
    wqkT = nc.declare_dram_parameter("wqkT", [C, 512], BF, isOutput=False)
    wvT = nc.declare_dram_parameter("wvT", [C, 256], BF, isOutput=False)
    projT = nc.declare_dram_parameter("projT", [256, C], BF, isOutput=False)
    outT = nc.declare_dram_parameter("outT", [C, N], F32, isOutput=True)

    with tile.TileContext(nc) as tc, ExitStack() as ctx:
        dram = ctx.enter_context(tc.tile_pool(name="dram", bufs=1, space="DRAM"))
        cst = ctx.enter_context(tc.tile_pool(name="cst", bufs=1))
        atn = ctx.enter_context(tc.tile_pool(name="atn", bufs=1))
        ps = ctx.enter_context(tc.tile_pool(name="ps", bufs=1, space="PSUM"))

        # ---- persistent SBUF tiles
        wqkT_s = cst.tile([128, 8, 512], BF)
        nc.gpsimd.dma_start(out=wqkT_s, in_=wqkT[:, :].rearrange("(kt p) c -> p kt c", p=128))
        xT_s = cst.tile([128, 8, N], BF)
        for qc in range(QC):
            for half in range(2):
                eng = nc.sync if half == 0 else nc.scalar
                eng.dma_start(
                    out=xT_s[:, half * 4:(half + 1) * 4, qc * 512:(qc + 1) * 512],
                    in_=xT[half * 512:(half + 1) * 512,
                           qc * 512:(qc + 1) * 512].rearrange(
                               "(kt p) n -> p kt n", p=128))
        wvT_s = cst.tile([128, 8, 256], BF)
        nc.gpsimd.dma_start(out=wvT_s, in_=wvT[:, :].rearrange("(kt p) c -> p kt c", p=128))
        projT_s = cst.tile([128, 2, C], BF)
        nc.gpsimd.dma_start(out=projT_s, in_=projT[:, :].rearrange("(hp p) c -> p hp c", p=128))

        kT_s = cst.tile([128, 2, N], BF)       # K^T per head pair
        qT_s = cst.tile([128, 2, N], BF)       # Q^T per head pair
        v_s = cst.tile([128, KT, 260], BF)     # V per head (4x65 blocks, col 64 = ones)
        nc.vector.memset(v_s, 1.0)
        attn_s = cst.tile([128, 2, N], BF)     # normalized O^T per pair

        exps = {}
        aos = {}

        def qk_chunk(dst, col, qc, nm):
            p_ps = ps.tile([128, 512], F32, tag="sc", bufs=2, name=f"qk_{nm}")
            for kt in range(8):
                nc.tensor.matmul(p_ps, wqkT_s[:, kt, col * 128:(col + 1) * 128],
                                 xT_s[:, kt, qc * 512:(qc + 1) * 512],
                                 start=(kt == 0), stop=(kt == 7))
            nc.vector.tensor_copy(dst, p_ps)

        def scores_g(b, g):
            qc, hp = BLOCKS[b]
            if g == 0:
                exps[b] = [atn.tile([128, KT, 512], BF, tag=f"exps{j}", bufs=2,
                                    name=f"exps{j}_{b}") for j in range(2)]
            sp = [ps.tile([128, 2, 512], F32, tag="sc", bufs=2,
                          name=f"sc_{b}_{g}_{j}") for j in range(2)]
            for jj in range(2):
                kt = 2 * g + jj
                for j in range(2):
                    nc.tensor.matmul(
                        sp[j][:, jj, :],
                        kT_s[j * 64:(j + 1) * 64, hp, kt * 128:(kt + 1) * 128],
                        qT_s[j * 64:(j + 1) * 64, hp, qc * 512:(qc + 1) * 512],
                        start=True, stop=True)
            for j in range(2):
                nc.scalar.activation(exps[b][j][:, 2 * g:2 * g + 2, :], sp[j],
                                     mybir.ActivationFunctionType.Exp, scale=0.125)

        def av_g(b, g):
            qc, hp = BLOCKS[b]
            if g == 0:
                aos[b] = [ps.tile([65, 512], F32, tag=f"ao{j}", bufs=2,
                                  name=f"ao_{b}_{j}") for j in range(2)]
            for jj in range(2):
                kt = 2 * g + jj
                for j in range(2):
                    h = 2 * hp + j
                    nc.tensor.matmul(aos[b][j], v_s[:, kt, h * 65:h * 65 + 65],
                                     exps[b][j][:, kt, :],
                                     start=(kt == 0), stop=(kt == KT - 1))

        def norm(b):
            qc, hp = BLOCKS[b]
            for j in range(2):
                ao = aos[b][j]
                # denominator -> DRAM -> [128,4] -> reciprocal -> DRAM -> [64,512] bcast
                den_s = atn.tile([1, 512], F32, tag="dens", bufs=2, name=f"den_{b}_{j}")
                nc.vector.tensor_copy(den_s, ao[64:65, :])
                dd = dram.tile([1, 512], F32, tag="rrd", bufs=4, name=f"dd_{b}_{j}")
                nc.gpsimd.dma_start(out=dd, in_=den_s)
                dt = atn.tile([128, 4], F32, tag="dt", bufs=2, name=f"dt_{b}_{j}")
                ddp = dd[:, :]
                nc.gpsimd.dma_start(out=dt, in_=bass.AP(
                    tensor=ddp.tensor, offset=ddp.offset, ap=[[1, 128], [128, 4]]))
                rt = atn.tile([128, 4], F32, tag="rt", bufs=2, name=f"rt_{b}_{j}")
                nc.vector.reciprocal(rt, dt)
                rd = dram.tile([1, 512], F32, tag="rtd", bufs=4, name=f"rd_{b}_{j}")
                rdp = rd[:, :]
                # store transposed: rt[p,k] -> rd[k*128+p], so rd is linear in q
                nc.gpsimd.dma_start(
                    out=bass.AP(tensor=rdp.tensor, offset=rdp.offset,
                                ap=[[1, 128], [128, 4]]),
                    in_=rt)
                rb = atn.tile([64, 512], F32, tag="rb", bufs=2, name=f"rb_{b}_{j}")
                nc.gpsimd.dma_start(out=rb, in_=bass.AP(
                    tensor=rdp.tensor, offset=rdp.offset,
                    ap=[[0, 64], [1, 512]]))
                if j == 0:
                    nc.vector.tensor_mul(attn_s[0:64, hp, qc * 512:(qc + 1) * 512],
                                         ao[0:64, :], rb)
                else:
                    tmp = atn.tile([64, 512], BF, tag="atmp", bufs=2, name=f"tmp_{b}")
                    nc.vector.tensor_mul(tmp, ao[0:64, :], rb)
                    nc.gpsimd.dma_start(
                        out=attn_s[64:128, hp, qc * 512:(qc + 1) * 512], in_=tmp)

        def proj_cts(qc, cts):
            for ct in cts:
                f_ps = ps.tile([128, 512], F32, tag="sc", bufs=2, name=f"f_{qc}_{ct}")
                for hp in range(2):
                    nc.tensor.matmul(f_ps, projT_s[:, hp, ct * 128:(ct + 1) * 128],
                                     attn_s[:, hp, qc * 512:(qc + 1) * 512],
                                     start=(hp == 0), stop=(hp == 1))
                f_s = atn.tile([128, 512], F32, tag="fs", bufs=4, name=f"fs_{qc}_{ct}")
                nc.vector.tensor_copy(f_s, f_ps)
                nc.sync.dma_start(
                    out=outT[ct * 128:(ct + 1) * 128, qc * 512:(qc + 1) * 512], in_=f_s)

        # ---- phase B interleaved with first two score blocks
        for qc in range(QC):
            qk_chunk(kT_s[:, 0, qc * 512:(qc + 1) * 512], 0, qc, f"k0{qc}")
        qk_chunk(qT_s[:, 0, 0:512], 2, 0, "q00")
        for g in range(8):
            scores_g(0, g)
        for qc in range(QC):
            qk_chunk(kT_s[:, 1, qc * 512:(qc + 1) * 512], 1, qc, f"k1{qc}")
        qk_chunk(qT_s[:, 1, 0:512], 3, 0, "q10")
        for g in range(8):
            scores_g(1, g)

        # ---- V (16 row tiles) with av(block 0) and Q(qc1) chunks interleaved
        for rt in range(KT):
            v_ps = ps.tile([128, 256], F32, tag="sc", bufs=2, name=f"v_{rt}")
            for kt in range(8):
                nc.tensor.matmul(v_ps, xT_s[:, kt, rt * 128:(rt + 1) * 128],
                                 wvT_s[:, kt, :], start=(kt == 0), stop=(kt == 7))
            vr = v_s[:, rt, 0:260]
            dst = bass.AP(tensor=vr.tensor, offset=vr.offset,
                          ap=[list(vr.ap[0]), [65, 4], [1, 64]])
            nc.vector.tensor_copy(dst, v_ps[:, :].rearrange("p (h e) -> p h e", h=4))
            if rt >= 8:
                av_g(0, rt - 8)
            if rt == 12:
                qk_chunk(qT_s[:, 0, 512:1024], 2, 1, "q01")
            if rt == 14:
                qk_chunk(qT_s[:, 1, 512:1024], 3, 1, "q11")
        norm(0)

        # ---- main loop: scores(b) + av(b-1) interleaved. Q chunks for later
        # blocks go into even blocks' g-loops; proj(qc) for a finished qc is
        # spread over g0-3 of the block after next (norm chain latency hidden).
        for b in range(2, len(BLOCKS)):
            for g in range(8):
                scores_g(b, g)
                av_g(b - 1, g)
                if b in (2, 4) and g in (4, 6):
                    nqc = b // 2 + 1
                    hp = 0 if g == 4 else 1
                    qk_chunk(qT_s[:, hp, nqc * 512:(nqc + 1) * 512],
                             2 + hp, nqc, f"q{hp}{nqc}")
                if b in (3, 5, 7) and g < 4:
                    pqc = (b - 3) // 2
                    proj_cts(pqc, [2 * g, 2 * g + 1])
            norm(b - 1)
        # ---- tail: av + norm of last block, final projection chunk
        for g in range(8):
            av_g(len(BLOCKS) - 1, g)
        norm(len(BLOCKS) - 1)
        proj_cts(QC - 1, range(8))
    _split_multi_waits(nc)
    return nc


def _split_multi_waits(nc):
    """This container's walrus supports one sync-wait per instruction; move
    extra waits onto preceding same-engine NoOps."""
    n_new = 0
    for bb in nc.m.functions[0].blocks:
        new = []
        for ins in bb.instructions:
            si = getattr(ins, "sync_info", None)
            ow = list(si.on_wait) if si is not None and si.on_wait else []
            if len(ow) > 1:
                for w in ow[:-1]:
                    n_new += 1
                    nop = mybir.InstNoOp(
                        name=f"{ins.name}_sw{n_new}",
                        engine=ins.engine,
                        sync_info=mybir.SyncInfo(on_wait=[w], on_update=[]),
                    )
                    new.append(nop)
                ins.sync_info = mybir.SyncInfo(
                    on_wait=[ow[-1]],
                    on_update=list(si.on_update) if si.on_update else [],
                )
            new.append(ins)
        bb.instructions = new


_NC = None
_LAST = None


def _ensure_ntff_hook():
    """The agent image's antenv lacks axon_hooks; shim it and register the
    ctypes NTFF profiler from trn_boot so trace=True yields exec_time_ns."""
    import sys
    import types
    try:
        import antenv.axon_hooks  # noqa: F401
        return
    except ImportError:
        pass
    mod = types.ModuleType("antenv.axon_hooks")
    holder = [None]
    mod.set_axon_ntff_profile_hook = lambda h: holder.__setitem__(0, h)
    mod.get_axon_ntff_profile_hook = lambda: holder[0]
    sys.modules["antenv.axon_hooks"] = mod
    import antenv
    antenv.axon_hooks = mod
    try:
        sys.path.insert(0, "/root/.axon_site")
        from trn_agent_boot.trn_boot import _ntff_profile_via_ctypes
        mod.set_axon_ntff_profile_hook(
            _ntff_profile_via_ctypes("/opt/axon/libaxon_pjrt.so"))
    except Exception:
        pass


def kernel(**inputs):
    global _NC, _LAST
    bf = ml_dtypes.bfloat16
    x = np.asarray(inputs["x"], np.float32)
    qkv_w = np.asarray(inputs["qkv_w"], np.float32)
    proj_w = np.asarray(inputs["proj_w"], np.float32)
    proj_b = np.asarray(inputs["proj_b"], np.float32)
    a1 = np.asarray(inputs["lora_w1_l1"], np.float32)
    b1 = np.asarray(inputs["lora_w1_l2"], np.float32)
    a2 = np.asarray(inputs["lora_w2_l1"], np.float32)
    b2 = np.asarray(inputs["lora_w2_l2"], np.float32)

    # fold LoRA into the dense weights (exact: x@W.T + (x@A.T)@B.T*2 = x@(W+2BA).T)
    Wqkv = qkv_w + 2.0 * (b1 @ a1)
    Wp = proj_w + 2.0 * (b2 @ a2)

    xTg = [np.ascontiguousarray(x[g].T).astype(bf) for g in range(B)]
    in_maps = []
    for c in range(8):
        g, hg = divmod(c, 4)
        r0 = hg * 256
        Kg = Wqkv[1024 + r0:1024 + r0 + 256]        # [256, 1024]
        Qg = Wqkv[r0:r0 + 256]
        Vg = Wqkv[2048 + r0:2048 + r0 + 256]
        m = {
            "xT": xTg[g],
            "wqkT": np.ascontiguousarray(np.vstack([Kg, Qg]).T).astype(bf),
            "wvT": np.ascontiguousarray(Vg.T).astype(bf),
            "projT": np.ascontiguousarray(Wp[:, r0:r0 + 256].T).astype(bf),
        }
        in_maps.append(m)

    if _NC is None:
        _NC = build()
    trace = os.environ.get("ATT_TRACE", "0") == "1"
    if trace:
        _ensure_ntff_hook()
    _LAST = run_bass_kernel_spmd(_NC, in_maps, core_ids=list(range(8)),
                                 trace=trace)
    out = np.empty((B, N, C), np.float32)
    for g in range(B):
        acc = np.zeros((C, N), np.float32)
        for hg in range(4):
            acc += np.asarray(_LAST.results[4 * g + hg]["outT"], np.float32)
        out[g] = acc.T + proj_b[None, :]
    return out


# revision 25
# speedup vs baseline: 1.0968x; 1.0968x over previous
"""Trainium2 Bass kernel: 16-head attention with LoRA (B=2, N=2048, C=1024).

v2: head-group sharding, no collectives. Core c handles batch c//4 and the
4 heads [4*(c%4), 4*(c%4)+4) over the FULL 2048-row sequence. LoRA is folded
into the qkv/proj weights on the host (W' = W + 2*B@A, exact). Each core
computes q,k,v for its heads, runs attention, and emits the partial output
projection over its 256 local channels; the host sums the 4 partials per
batch and adds the bias.

Pipeline: per (query-chunk, head-pair) block, scores (PE, row-packed K=64
pairs) feed exp (Scalar) feed attnV (PE, interleaved one block behind), so
ScalarE's exp stream and the PE matmul stream overlap continuously.
"""

import os
from contextlib import ExitStack

import numpy as np
import ml_dtypes

import concourse.bass as bass
import concourse.mybir as mybir
import concourse.tile as tile
from concourse.bass_utils import run_bass_kernel_spmd

B, N, C, H, D = 2, 2048, 1024, 16, 64
KT = N // 128    # 16 kv tiles of 128
QC = 4           # query chunks of 512
BF = mybir.dt.bfloat16
F32 = mybir.dt.float32
BLOCKS = [(qc, hp) for qc in range(QC) for hp in range(2)]


def build():
    nc = bass.Bass()
    xT = nc.declare_dram_parameter("xT", [C, N], BF, isOutput=False)
    wqkT = nc.declare_dram_parameter("wqkT", [C, 512], BF, isOutput=False)
    wvT = nc.declare_dram_parameter("wvT", [C, 256], BF, isOutput=False)
    projT = nc.declare_dram_parameter("projT", [256, C], BF, isOutput=False)
    outT = nc.declare_dram_parameter("outT", [C, N], F32, isOutput=True)

    with tile.TileContext(nc) as tc, ExitStack() as ctx:
        dram = ctx.enter_context(tc.tile_pool(name="dram", bufs=1, space="DRAM"))
        cst = ctx.enter_context(tc.tile_pool(name="cst", bufs=1))
        atn = ctx.enter_context(tc.tile_pool(name="atn", bufs=1))
        ps = ctx.enter_context(tc.tile_pool(name="ps", bufs=1, space="PSUM"))

        # ---- persistent SBUF tiles
        wqkT_s = cst.tile([128, 8, 512], BF)
        nc.gpsimd.dma_start(out=wqkT_s, in_=wqkT[:, :].rearrange("(kt p) c -> p kt c", p=128))
        xT_s = cst.tile([128, 8, N], BF)
        for qc in range(QC):
            for half in range(2):
                eng = nc.sync if half == 0 else nc.scalar
                eng.dma_start(
                    out=xT_s[:, half * 4:(half + 1) * 4, qc * 512:(qc + 1) * 512],
                    in_=xT[half * 512:(half + 1) * 512,
                           qc * 512:(qc + 1) * 512].rearrange(
                               "(kt p) n -> p kt n", p=128))
        wvT_s = cst.tile([128, 8, 256], BF)
        nc.gpsimd.dma_start(out=wvT_s, in_=wvT[:, :].rearrange("(kt p) c -> p kt c", p=128))
        projT_s = cst.tile([128, 2, C], BF)
        nc.gpsimd.dma_start(out=projT_s, in_=projT[:, :].rearrange("(hp p) c -> p hp c", p=128))

        kT_s = cst.tile([128, 2, N], BF)       # K^T per head pair
        qT_s = cst.tile([128, 2, N], BF)       # Q^T per head pair
        v_s = cst.tile([128, KT, 260], BF)     # V per head (4x65 blocks, col 64 = ones)
        nc.vector.memset(v_s, 1.0)
        attn_s = cst.tile([128, 2, N], BF)     # normalized O^T per pair

        exps = {}
        aos = {}

        def qk_chunk(dst, col, qc, nm):
            p_ps = ps.tile([128, 512], F32, tag="sc", bufs=2, name=f"qk_{nm}")
            for kt in range(8):
                nc.tensor.matmul(p_ps, wqkT_s[:, kt, col * 128:(col + 1) * 128],
                                 xT_s[:, kt, qc * 512:(qc + 1) * 512],
                                 start=(kt == 0), stop=(kt == 7))
            nc.vector.tensor_copy(dst, p_ps)

        def scores_g(b, g):
            qc, hp = BLOCKS[b]
            if g == 0:
                exps[b] = [atn.tile([128, KT, 512], BF, tag=f"exps{j}", bufs=2,
                                    name=f"exps{j}_{b}") for j in range(2)]
            sp = [ps.tile([128, 2, 512], F32, tag="sc", bufs=2,
                          name=f"sc_{b}_{g}_{j}") for j in range(2)]
            for jj in range(2):
                kt = 2 * g + jj
                for j in range(2):
                    nc.tensor.matmul(
                        sp[j][:, jj, :],
                        kT_s[j * 64:(j + 1) * 64, hp, kt * 128:(kt + 1) * 128],
                        qT_s[j * 64:(j + 1) * 64, hp, qc * 512:(qc + 1) * 512],
                        start=True, stop=True)
            for j in range(2):
                nc.scalar.activation(exps[b][j][:, 2 * g:2 * g + 2, :], sp[j],
                                     mybir.ActivationFunctionType.Exp, scale=0.125)

        def av_g(b, g):
            qc, hp = BLOCKS[b]
            if g == 0:
                aos[b] = [ps.tile([65, 512], F32, tag=f"ao{j}", bufs=2,
                                  name=f"ao_{b}_{j}") for j in range(2)]
            for jj in range(2):
                kt = 2 * g + jj
                for j in range(2):
                    h = 2 * hp + j
                    nc.tensor.matmul(aos[b][j], v_s[:, kt, h * 65:h * 65 + 65],
                                     exps[b][j][:, kt, :],
                                     start=(kt == 0), stop=(kt == KT - 1))

        def norm(b):
            qc, hp = BLOCKS[b]
            # both heads' denominators batched: [2,512] -> DRAM -> [128,8]
            # -> reciprocal -> DRAM (transposed) -> two [64,512] broadcasts
            den_s = atn.tile([33, 512], F32, tag="dens", bufs=2, name=f"den_{b}")
            for j in range(2):
                nc.vector.tensor_copy(den_s[32 * j:32 * j + 1, :],
                                      aos[b][j][64:65, :])
            dd = dram.tile([2, 512], F32, tag="rrd", bufs=3, name=f"dd_{b}")
            dsp = den_s[:, :]
            nc.gpsimd.dma_start(out=dd, in_=bass.AP(
                tensor=dsp.tensor, offset=dsp.offset,
                ap=[[dsp.ap[0][0] * 32, 2], [1, 512]]))
            dt = atn.tile([128, 2, 4], F32, tag="dt", bufs=2, name=f"dt_{b}")
            ddp = dd[:, :]
            nc.gpsimd.dma_start(out=dt, in_=bass.AP(
                tensor=ddp.tensor, offset=ddp.offset,
                ap=[[1, 128], [512, 2], [128, 4]]))
            rt = atn.tile([128, 2, 4], F32, tag="rt", bufs=2, name=f"rt_{b}")
            nc.vector.reciprocal(rt, dt)
            rd = dram.tile([2, 512], F32, tag="rtd", bufs=3, name=f"rd_{b}")
            rdp = rd[:, :]
            nc.gpsimd.dma_start(
                out=bass.AP(tensor=rdp.tensor, offset=rdp.offset,
                            ap=[[1, 128], [512, 2], [128, 4]]),
                in_=rt)
            for j in range(2):
                rb = atn.tile([64, 512], F32, tag="rb", bufs=2, name=f"rb_{b}_{j}")
                nc.gpsimd.dma_start(out=rb, in_=bass.AP(
                    tensor=rdp.tensor, offset=rdp.offset + j * 512,
                    ap=[[0, 64], [1, 512]]))
                if j == 0:
                    nc.vector.tensor_mul(attn_s[0:64, hp, qc * 512:(qc + 1) * 512],
                                         aos[b][j][0:64, :], rb)
                else:
                    tmp = atn.tile([64, 512], BF, tag="atmp", bufs=2, name=f"tmp_{b}")
                    nc.vector.tensor_mul(tmp, aos[b][j][0:64, :], rb)
                    nc.gpsimd.dma_start(
                        out=attn_s[64:128, hp, qc * 512:(qc + 1) * 512], in_=tmp)

        def proj_cts(qc, cts):
            for ct in cts:
                f_ps = ps.tile([128, 512], F32, tag="sc", bufs=2, name=f"f_{qc}_{ct}")
                for hp in range(2):
                    nc.tensor.matmul(f_ps, projT_s[:, hp, ct * 128:(ct + 1) * 128],
                                     attn_s[:, hp, qc * 512:(qc + 1) * 512],
                                     start=(hp == 0), stop=(hp == 1))
                f_s = atn.tile([128, 512], F32, tag="fs", bufs=4, name=f"fs_{qc}_{ct}")
                nc.vector.tensor_copy(f_s, f_ps)
                nc.sync.dma_start(
                    out=outT[ct * 128:(ct + 1) * 128, qc * 512:(qc + 1) * 512], in_=f_s)

        def v_chunk(rt):
            v_ps = ps.tile([128, 256], F32, tag="sc", bufs=2, name=f"v_{rt}")
            for kt in range(8):
                nc.tensor.matmul(v_ps, xT_s[:, kt, rt * 128:(rt + 1) * 128],
                                 wvT_s[:, kt, :], start=(kt == 0), stop=(kt == 7))
            vr = v_s[:, rt, 0:260]
            dst = bass.AP(tensor=vr.tensor, offset=vr.offset,
                          ap=[list(vr.ap[0]), [65, 4], [1, 64]])
            nc.vector.tensor_copy(dst, v_ps[:, :].rearrange("p (h e) -> p h e", h=4))

        # ---- phase B: K and first Q chunks (paced by the x input DMAs)
        for qc in range(QC):
            qk_chunk(kT_s[:, 0, qc * 512:(qc + 1) * 512], 0, qc, f"k0{qc}")
        qk_chunk(qT_s[:, 0, 0:512], 2, 0, "q00")
        for qc in range(QC):
            qk_chunk(kT_s[:, 1, qc * 512:(qc + 1) * 512], 1, qc, f"k1{qc}")
        qk_chunk(qT_s[:, 1, 0:512], 3, 0, "q10")

        # ---- prologue: blocks 0+1 scores exp-paced, V chunks fill the PE,
        # av(block 0) trails one group behind its exps
        for g in range(8):
            scores_g(0, g)
            v_chunk(2 * g)
            scores_g(1, g)
            v_chunk(2 * g + 1)
            if g >= 1:
                av_g(0, g - 1)
        av_g(0, 7)
        qk_chunk(qT_s[:, 0, 512:1024], 2, 1, "q01")
        qk_chunk(qT_s[:, 1, 512:1024], 3, 1, "q11")
        norm(0)

        # ---- main loop: scores(b) + av(b-1) interleaved. Q chunks for later
        # qcs injected at g4; proj(qc) spread over g0-3 two blocks after its
        # last norm (hides the reciprocal chain); last block's av trails by
        # one group so the tail is short.
        last = len(BLOCKS) - 1
        for b in range(2, len(BLOCKS)):
            for g in range(8):
                scores_g(b, g)
                av_g(b - 1, g)
                if b == last and g >= 1:
                    av_g(last, g - 1)
                if b in (2, 3) and g == 4:
                    hp = b - 2
                    qk_chunk(qT_s[:, hp, 1024:1536], 2 + hp, 2, f"q{hp}2")
                if b in (4, 5) and g == 4:
                    hp = b - 4
                    qk_chunk(qT_s[:, hp, 1536:2048], 2 + hp, 3, f"q{hp}3")
                if b in (3, 5, 7) and g < 4:
                    pqc = (b - 3) // 2
                    proj_cts(pqc, [2 * g, 2 * g + 1])
            norm(b - 1)
        # ---- tail
        av_g(last, 7)
        norm(last)
        proj_cts(QC - 1, range(8))
    _split_multi_waits(nc)
    return nc


def _split_multi_waits(nc):
    """This container's walrus supports one sync-wait per instruction; move
    extra waits onto preceding same-engine NoOps."""
    n_new = 0
    for bb in nc.m.functions[0].blocks:
        new = []
        for ins in bb.instructions:
            si = getattr(ins, "sync_info", None)
            ow = list(si.on_wait) if si is not None and si.on_wait else []
            if len(ow) > 1:
                for w in ow[:-1]:
                    n_new += 1
                    nop = mybir.InstNoOp(
                        name=f"{ins.name}_sw{n_new}",
                        engine=ins.engine,
                        sync_info=mybir.SyncInfo(on_wait=[w], on_update=[]),
                    )
                    new.append(nop)
                ins.sync_info = mybir.SyncInfo(
                    on_wait=[ow[-1]],
                    on_update=list(si.on_update) if si.on_update else [],
                )
            new.append(ins)
        bb.instructions = new


_NC = None
_LAST = None


def _ensure_ntff_hook():
    """The agent image's antenv lacks axon_hooks; shim it and register the
    ctypes NTFF profiler from trn_boot so trace=True yields exec_time_ns."""
    import sys
    import types
    try:
        import antenv.axon_hooks  # noqa: F401
        return
    except ImportError:
        pass
    mod = types.ModuleType("antenv.axon_hooks")
    holder = [None]
    mod.set_axon_ntff_profile_hook = lambda h: holder.__setitem__(0, h)
    mod.get_axon_ntff_profile_hook = lambda: holder[0]
    sys.modules["antenv.axon_hooks"] = mod
    import antenv
    antenv.axon_hooks = mod
    try:
        sys.path.insert(0, "/root/.axon_site")
        from trn_agent_boot.trn_boot import _ntff_profile_via_ctypes
        mod.set_axon_ntff_profile_hook(
            _ntff_profile_via_ctypes("/opt/axon/libaxon_pjrt.so"))
    except Exception:
        pass


def kernel(**inputs):
    global _NC, _LAST
    bf = ml_dtypes.bfloat16
    x = np.asarray(inputs["x"], np.float32)
    qkv_w = np.asarray(inputs["qkv_w"], np.float32)
    proj_w = np.asarray(inputs["proj_w"], np.float32)
    proj_b = np.asarray(inputs["proj_b"], np.float32)
    a1 = np.asarray(inputs["lora_w1_l1"], np.float32)
    b1 = np.asarray(inputs["lora_w1_l2"], np.float32)
    a2 = np.asarray(inputs["lora_w2_l1"], np.float32)
    b2 = np.asarray(inputs["lora_w2_l2"], np.float32)

    # fold LoRA into the dense weights (exact: x@W.T + (x@A.T)@B.T*2 = x@(W+2BA).T)
    Wqkv = qkv_w + 2.0 * (b1 @ a1)
    Wp = proj_w + 2.0 * (b2 @ a2)

    xTg = [np.ascontiguousarray(x[g].T).astype(bf) for g in range(B)]
    in_maps = []
    for c in range(8):
        g, hg = divmod(c, 4)
        r0 = hg * 256
        Kg = Wqkv[1024 + r0:1024 + r0 + 256]        # [256, 1024]
        Qg = Wqkv[r0:r0 + 256]
        Vg = Wqkv[2048 + r0:2048 + r0 + 256]
        m = {
            "xT": xTg[g],
            "wqkT": np.ascontiguousarray(np.vstack([Kg, Qg]).T).astype(bf),
            "wvT": np.ascontiguousarray(Vg.T).astype(bf),
            "projT": np.ascontiguousarray(Wp[:, r0:r0 + 256].T).astype(bf),
        }
        in_maps.append(m)

    if _NC is None:
        _NC = build()
    trace = os.environ.get("ATT_TRACE", "0") == "1"
    if trace:
        _ensure_ntff_hook()
    _LAST = run_bass_kernel_spmd(_NC, in_maps, core_ids=list(range(8)),
                                 trace=trace)
    out = np.empty((B, N, C), np.float32)
    for g in range(B):
        acc = np.zeros((C, N), np.float32)
        for hg in range(4):
            acc += np.asarray(_LAST.results[4 * g + hg]["outT"], np.float32)
        out[g] = acc.T + proj_b[None, :]
    return out


# revision 32
# speedup vs baseline: 1.1760x; 1.0721x over previous
"""Trainium2 Bass kernel: 16-head attention with LoRA (B=2, N=2048, C=1024).

v2: head-group sharding, no collectives. Core c handles batch c//4 and the
4 heads [4*(c%4), 4*(c%4)+4) over the FULL 2048-row sequence. LoRA is folded
into the qkv/proj weights on the host (W' = W + 2*B@A, exact). Each core
computes q,k,v for its heads, runs attention, and emits the partial output
projection over its 256 local channels; the host sums the 4 partials per
batch and adds the bias.

Pipeline: per (query-chunk, head-pair) block, scores (PE, row-packed K=64
pairs) feed exp (Scalar) feed attnV (PE, interleaved one block behind), so
ScalarE's exp stream and the PE matmul stream overlap continuously.
"""

import os
from contextlib import ExitStack

import numpy as np
import ml_dtypes

import concourse.bass as bass
import concourse.mybir as mybir
import concourse.tile as tile
from concourse.bass_utils import run_bass_kernel_spmd

B, N, C, H, D = 2, 2048, 1024, 16, 64
KT = N // 128    # 16 kv tiles of 128
QC = 4           # query chunks of 512
BF = mybir.dt.bfloat16
F32 = mybir.dt.float32
BLOCKS = [(qc, hp) for qc in range(QC) for hp in range(2)]


def build():
    nc = bass.Bass()
    xT = nc.declare_dram_parameter("xT", [C, N], BF, isOutput=False)
    wqkT = nc.declare_dram_parameter("wqkT", [C, 512], BF, isOutput=False)
    wvT = nc.declare_dram_parameter("wvT", [C, 256], BF, isOutput=False)
    projT = nc.declare_dram_parameter("projT", [256, C], BF, isOutput=False)
    outT = nc.declare_dram_parameter("outT", [C, N], F32, isOutput=True)

    with tile.TileContext(nc) as tc, ExitStack() as ctx:
        dram = ctx.enter_context(tc.tile_pool(name="dram", bufs=1, space="DRAM"))
        cst = ctx.enter_context(tc.tile_pool(name="cst", bufs=1))
        atn = ctx.enter_context(tc.tile_pool(name="atn", bufs=1))
        ps = ctx.enter_context(tc.tile_pool(name="ps", bufs=1, space="PSUM"))

        # ---- persistent SBUF tiles
        wqkT_s = cst.tile([128, 8, 512], BF)
        nc.gpsimd.dma_start(out=wqkT_s, in_=wqkT[:, :].rearrange("(kt p) c -> p kt c", p=128))
        xT_s = cst.tile([128, 8, N], BF)
        x_engs = [nc.sync, nc.scalar, nc.gpsimd]
        for qc in range(QC):
            for half in range(2):
                eng = x_engs[(2 * qc + half) % 3]
                eng.dma_start(
                    out=xT_s[:, half * 4:(half + 1) * 4, qc * 512:(qc + 1) * 512],
                    in_=xT[half * 512:(half + 1) * 512,
                           qc * 512:(qc + 1) * 512].rearrange(
                               "(kt p) n -> p kt n", p=128))
        wvT_s = cst.tile([128, 8, 256], BF)
        nc.gpsimd.dma_start(out=wvT_s, in_=wvT[:, :].rearrange("(kt p) c -> p kt c", p=128))
        projT_s = cst.tile([128, 2, C], BF)
        nc.gpsimd.dma_start(out=projT_s, in_=projT[:, :].rearrange("(hp p) c -> p hp c", p=128))

        kT_s = cst.tile([128, 2, N], BF)       # K^T per head pair
        qT_s = cst.tile([128, 2, N], BF)       # Q^T per head pair
        v_s = cst.tile([128, KT, 260], BF)     # V per head (4x65 blocks, col 64 = ones)
        nc.vector.memset(v_s, 1.0)
        attn_s = cst.tile([128, 2, N], BF)     # normalized O^T per pair

        exps = {}
        aos = {}

        def qk_chunk(dst, col, qc, nm):
            p_ps = ps.tile([128, 512], F32, tag="sc", bufs=4, name=f"qk_{nm}")
            for kt in range(8):
                nc.tensor.matmul(p_ps, wqkT_s[:, kt, col * 128:(col + 1) * 128],
                                 xT_s[:, kt, qc * 512:(qc + 1) * 512],
                                 start=(kt == 0), stop=(kt == 7))
            nc.vector.tensor_copy(dst, p_ps)

        def scores_g(b, g):
            qc, hp = BLOCKS[b]
            if g == 0:
                exps[b] = [atn.tile([128, KT, 512], BF, tag=f"exps{j}", bufs=2,
                                    name=f"exps{j}_{b}") for j in range(2)]
            for jj in range(2):
                kt = 2 * g + jj
                sp = [ps.tile([128, 512], F32, tag="sc", bufs=4,
                              name=f"sc_{b}_{kt}_{j}") for j in range(2)]
                for j in range(2):
                    nc.tensor.matmul(
                        sp[j],
                        kT_s[j * 64:(j + 1) * 64, hp, kt * 128:(kt + 1) * 128],
                        qT_s[j * 64:(j + 1) * 64, hp, qc * 512:(qc + 1) * 512],
                        start=True, stop=True)
                for j in range(2):
                    nc.scalar.activation(exps[b][j][:, kt, :], sp[j],
                                         mybir.ActivationFunctionType.Exp,
                                         scale=0.125)

        def av_g(b, g):
            qc, hp = BLOCKS[b]
            if g == 0:
                aos[b] = [ps.tile([65, 512], F32, tag=f"ao{j}", bufs=2,
                                  name=f"ao_{b}_{j}") for j in range(2)]
            for jj in range(2):
                kt = 2 * g + jj
                for j in range(2):
                    h = 2 * hp + j
                    nc.tensor.matmul(aos[b][j], v_s[:, kt, h * 65:h * 65 + 65],
                                     exps[b][j][:, kt, :],
                                     start=(kt == 0), stop=(kt == KT - 1))

        def norm(b):
            qc, hp = BLOCKS[b]
            # both heads' denominators batched: [2,512] -> DRAM -> [128,8]
            # -> reciprocal -> DRAM (transposed) -> two [64,512] broadcasts
            den_s = atn.tile([33, 512], F32, tag="dens", bufs=2, name=f"den_{b}")
            for j in range(2):
                nc.vector.tensor_copy(den_s[32 * j:32 * j + 1, :],
                                      aos[b][j][64:65, :])
            dd = dram.tile([2, 512], F32, tag="rrd", bufs=3, name=f"dd_{b}")
            dsp = den_s[:, :]
            nc.gpsimd.dma_start(out=dd, in_=bass.AP(
                tensor=dsp.tensor, offset=dsp.offset,
                ap=[[dsp.ap[0][0] * 32, 2], [1, 512]]))
            dt = atn.tile([128, 2, 4], F32, tag="dt", bufs=2, name=f"dt_{b}")
            ddp = dd[:, :]
            nc.gpsimd.dma_start(out=dt, in_=bass.AP(
                tensor=ddp.tensor, offset=ddp.offset,
                ap=[[1, 128], [512, 2], [128, 4]]))
            rt = atn.tile([128, 2, 4], F32, tag="rt", bufs=2, name=f"rt_{b}")
            nc.vector.reciprocal(rt, dt)
            rd = dram.tile([2, 512], F32, tag="rtd", bufs=3, name=f"rd_{b}")
            rdp = rd[:, :]
            nc.gpsimd.dma_start(
                out=bass.AP(tensor=rdp.tensor, offset=rdp.offset,
                            ap=[[1, 128], [512, 2], [128, 4]]),
                in_=rt)
            for j in range(2):
                rb = atn.tile([64, 512], F32, tag="rb", bufs=2, name=f"rb_{b}_{j}")
                nc.gpsimd.dma_start(out=rb, in_=bass.AP(
                    tensor=rdp.tensor, offset=rdp.offset + j * 512,
                    ap=[[0, 64], [1, 512]]))
                if j == 0:
                    nc.vector.tensor_mul(attn_s[0:64, hp, qc * 512:(qc + 1) * 512],
                                         aos[b][j][0:64, :], rb)
                else:
                    tmp = atn.tile([64, 512], BF, tag="atmp", bufs=2, name=f"tmp_{b}")
                    nc.vector.tensor_mul(tmp, aos[b][j][0:64, :], rb)
                    nc.gpsimd.dma_start(
                        out=attn_s[64:128, hp, qc * 512:(qc + 1) * 512], in_=tmp)

        def proj_cts(qc, cts):
            for ct in cts:
                f_ps = ps.tile([128, 512], F32, tag="sc", bufs=4, name=f"f_{qc}_{ct}")
                for hp in range(2):
                    nc.tensor.matmul(f_ps, projT_s[:, hp, ct * 128:(ct + 1) * 128],
                                     attn_s[:, hp, qc * 512:(qc + 1) * 512],
                                     start=(hp == 0), stop=(hp == 1))
                f_s = atn.tile([128, 512], F32, tag="fs", bufs=4, name=f"fs_{qc}_{ct}")
                nc.vector.tensor_copy(f_s, f_ps)
                nc.sync.dma_start(
                    out=outT[ct * 128:(ct + 1) * 128, qc * 512:(qc + 1) * 512], in_=f_s)

        def v_chunk(rt):
            v_ps = ps.tile([128, 256], F32, tag="sc", bufs=4, name=f"v_{rt}")
            for kt in range(8):
                nc.tensor.matmul(v_ps, xT_s[:, kt, rt * 128:(rt + 1) * 128],
                                 wvT_s[:, kt, :], start=(kt == 0), stop=(kt == 7))
            vr = v_s[:, rt, 0:260]
            dst = bass.AP(tensor=vr.tensor, offset=vr.offset,
                          ap=[list(vr.ap[0]), [65, 4], [1, 64]])
            nc.vector.tensor_copy(dst, v_ps[:, :].rearrange("p (h e) -> p h e", h=4))

        # ---- phase B: K and first Q chunks (paced by the x input DMAs)
        for qc in range(QC):
            qk_chunk(kT_s[:, 0, qc * 512:(qc + 1) * 512], 0, qc, f"k0{qc}")
        qk_chunk(qT_s[:, 0, 0:512], 2, 0, "q00")
        for qc in range(QC):
            qk_chunk(kT_s[:, 1, qc * 512:(qc + 1) * 512], 1, qc, f"k1{qc}")
        qk_chunk(qT_s[:, 1, 0:512], 3, 0, "q10")

        # ---- prologue: blocks 0+1 scores exp-paced, V chunks fill the PE,
        # av(block 0) trails one group behind its exps
        for g in range(8):
            scores_g(0, g)
            v_chunk(2 * g)
            scores_g(1, g)
            v_chunk(2 * g + 1)
            if g >= 1:
                av_g(0, g - 1)
        av_g(0, 7)
        qk_chunk(qT_s[:, 0, 512:1024], 2, 1, "q01")
        qk_chunk(qT_s[:, 1, 512:1024], 3, 1, "q11")
        norm(0)

        # ---- main loop: scores(b) + av(b-1) interleaved. Q chunks for later
        # qcs injected at g4; proj(qc) spread over g0-3 two blocks after its
        # last norm (hides the reciprocal chain); last block's av trails by
        # one group so the tail is short.
        last = len(BLOCKS) - 1
        for b in range(2, len(BLOCKS)):
            for g in range(8):
                scores_g(b, g)
                av_g(b - 1, g)
                if b == last and g >= 1:
                    av_g(last, g - 1)
                if b in (2, 3) and g == 4:
                    hp = b - 2
                    qk_chunk(qT_s[:, hp, 1024:1536], 2 + hp, 2, f"q{hp}2")
                if b in (4, 5) and g == 4:
                    hp = b - 4
                    qk_chunk(qT_s[:, hp, 1536:2048], 2 + hp, 3, f"q{hp}3")
                if b in (3, 5) and g < 4:
                    pqc = (b - 3) // 2
                    proj_cts(pqc, [2 * g, 2 * g + 1])
            norm(b - 1)
        # ---- tail: last av group, fast direct-reciprocal norm for the last
        # block, with proj(qc2) filling the reciprocal-chain latency
        av_g(last, 7)
        qc, hp = BLOCKS[last]
        rrs = []
        for j in range(2):
            rr = atn.tile([1, 512], F32, tag="rrf", bufs=2, name=f"rrf_{j}")
            nc.vector.reciprocal(rr, aos[last][j][64:65, :])
            rrd = dram.tile([1, 512], F32, tag="rrfd", bufs=2, name=f"rrfd_{j}")
            nc.gpsimd.dma_start(out=rrd, in_=rr)
            rrs.append(rrd)
        proj_cts(QC - 2, range(8))
        for j in range(2):
            rdp = rrs[j][:, :]
            rb = atn.tile([64, 512], F32, tag="rb", bufs=2, name=f"rbf_{j}")
            nc.gpsimd.dma_start(out=rb, in_=bass.AP(
                tensor=rdp.tensor, offset=rdp.offset, ap=[[0, 64], [1, 512]]))
            if j == 0:
                nc.vector.tensor_mul(attn_s[0:64, hp, qc * 512:(qc + 1) * 512],
                                     aos[last][j][0:64, :], rb)
            else:
                tmp = atn.tile([64, 512], BF, tag="atmp", bufs=2, name="tmpf")
                nc.vector.tensor_mul(tmp, aos[last][j][0:64, :], rb)
                nc.gpsimd.dma_start(
                    out=attn_s[64:128, hp, qc * 512:(qc + 1) * 512], in_=tmp)
        proj_cts(QC - 1, range(8))
    _split_multi_waits(nc)
    return nc


def _split_multi_waits(nc):
    """This container's walrus supports one sync-wait per instruction; move
    extra waits onto preceding same-engine NoOps."""
    n_new = 0
    for bb in nc.m.functions[0].blocks:
        new = []
        for ins in bb.instructions:
            si = getattr(ins, "sync_info", None)
            ow = list(si.on_wait) if si is not None and si.on_wait else []
            if len(ow) > 1:
                for w in ow[:-1]:
                    n_new += 1
                    nop = mybir.InstNoOp(
                        name=f"{ins.name}_sw{n_new}",
                        engine=ins.engine,
                        sync_info=mybir.SyncInfo(on_wait=[w], on_update=[]),
                    )
                    new.append(nop)
                ins.sync_info = mybir.SyncInfo(
                    on_wait=[ow[-1]],
                    on_update=list(si.on_update) if si.on_update else [],
                )
            new.append(ins)
        bb.instructions = new


_NC = None
_LAST = None


def _ensure_ntff_hook():
    """The agent image's antenv lacks axon_hooks; shim it and register the
    ctypes NTFF profiler from trn_boot so trace=True yields exec_time_ns."""
    import sys
    import types
    try:
        import antenv.axon_hooks  # noqa: F401
        return
    except ImportError:
        pass
    mod = types.ModuleType("antenv.axon_hooks")
    holder = [None]
    mod.set_axon_ntff_profile_hook = lambda h: holder.__setitem__(0, h)
    mod.get_axon_ntff_profile_hook = lambda: holder[0]
    sys.modules["antenv.axon_hooks"] = mod
    import antenv
    antenv.axon_hooks = mod
    try:
        sys.path.insert(0, "/root/.axon_site")
        from trn_agent_boot.trn_boot import _ntff_profile_via_ctypes
        mod.set_axon_ntff_profile_hook(
            _ntff_profile_via_ctypes("/opt/axon/libaxon_pjrt.so"))
    except Exception:
        pass


def kernel(**inputs):
    global _NC, _LAST
    bf = ml_dtypes.bfloat16
    x = np.asarray(inputs["x"], np.float32)
    qkv_w = np.asarray(inputs["qkv_w"], np.float32)
    proj_w = np.asarray(inputs["proj_w"], np.float32)
    proj_b = np.asarray(inputs["proj_b"], np.float32)
    a1 = np.asarray(inputs["lora_w1_l1"], np.float32)
    b1 = np.asarray(inputs["lora_w1_l2"], np.float32)
    a2 = np.asarray(inputs["lora_w2_l1"], np.float32)
    b2 = np.asarray(inputs["lora_w2_l2"], np.float32)

    # fold LoRA into the dense weights (exact: x@W.T + (x@A.T)@B.T*2 = x@(W+2BA).T)
    Wqkv = qkv_w + 2.0 * (b1 @ a1)
    Wp = proj_w + 2.0 * (b2 @ a2)

    xTg = [np.ascontiguousarray(x[g].T).astype(bf) for g in range(B)]
    in_maps = []
    for c in range(8):
        g, hg = divmod(c, 4)
        r0 = hg * 256
        Kg = Wqkv[1024 + r0:1024 + r0 + 256]        # [256, 1024]
        Qg = Wqkv[r0:r0 + 256]
        Vg = Wqkv[2048 + r0:2048 + r0 + 256]
        m = {
            "xT": xTg[g],
            "wqkT": np.ascontiguousarray(np.vstack([Kg, Qg]).T).astype(bf),
            "wvT": np.ascontiguousarray(Vg.T).astype(bf),
            "projT": np.ascontiguousarray(Wp[:, r0:r0 + 256].T).astype(bf),
        }
        in_maps.append(m)

    if _NC is None:
        _NC = build()
    trace = os.environ.get("ATT_TRACE", "0") == "1"
    if trace:
        _ensure_ntff_hook()
    _LAST = run_bass_kernel_spmd(_NC, in_maps, core_ids=list(range(8)),
                                 trace=trace)
    out = np.empty((B, N, C), np.float32)
    for g in range(B):
        acc = np.zeros((C, N), np.float32)
        for hg in range(4):
            acc += np.asarray(_LAST.results[4 * g + hg]["outT"], np.float32)
        out[g] = acc.T + proj_b[None, :]
    return out
